# revision 1
# baseline (speedup 1.0000x reference)
"""v5: v4 + two PE-stream cuts.

  * A and X1 fused: one matmul per (chunk, col-group) with the 16-wide
    stationary [gh | gl*256] -> out [16, 512] (A in rows 0:8, X1*256 in
    rows 8:16 of the same bank).  512 GEMM matmuls instead of 768.
  * The descale/combine moves into the transpose stage: plain matmuls
    with a [16,8] matrix M = [I; 2^-8*I] contract the 16 A|X1 rows into
    transposed-and-combined [token, expert] tiles, and a second
    accumulating matmul adds X2 via 2^-8*I.  Bias is added by the DVE
    copy out of the transpose bank (tensor_tensor with a replicated
    bias tile).
  * hh streamed in 8 MiB loads.
"""

import numpy as np
import ml_dtypes

import concourse.bass as bass
import concourse.tile as tile
from concourse import bacc, mybir
from concourse.bass_utils import run_bass_kernel_spmd

F32 = mybir.dt.float32
BF16 = mybir.dt.bfloat16
F8 = mybir.dt.float8e4
U32 = mybir.dt.uint32

N_CORES = 8
B, S, D, E = 4, 8192, 4096, 8
T_TOTAL = B * S
T_CORE = T_TOTAL // N_CORES
P = 128
NCH = D // P                       # 32
T_HALF = T_CORE // 2               # 2048
N_Q = T_HALF // 512                # 4
N_BJ = 4
N_BLK = N_Q * N_BJ                 # 16
CQH = 16                           # bf16 chunks per DMA (8 MiB)
CQL = 32                           # fp8 chunks per DMA (8 MiB, one per half)
LO_SCALE = 256.0
INV_LO = 1.0 / LO_SCALE

_NC_CACHE = {}

TRACE = False
LAST_RESULT = None


def build_router_nc(n_rep=1, hbufs=2, lbufs=1):
    nc = bacc.Bacc(None, target_bir_lowering=False)

    hh = nc.dram_tensor("hh", [2, P, NCH, T_HALF], BF16,
                        kind="ExternalInput")
    hl = nc.dram_tensor("hl", [2, P, NCH, T_HALF], F8, kind="ExternalInput")
    g2 = nc.dram_tensor("g2", [P, NCH, 2 * E], BF16, kind="ExternalInput")
    # combine-matrix tiles, host-prebuilt: [:,0,:] = [I; 2^-8 I] pattern at
    # each 32q base (A|X1 contraction), [:,1,:] = 2^-8 I at 32q (X2)
    mm = nc.dram_tensor("mm", [P, 2, E], F32, kind="ExternalInput")
    bt = nc.dram_tensor("bt", [P, N_BLK, E], F32, kind="ExternalInput")
    ow = nc.dram_tensor("ow", [2, P, N_Q, N_BJ, 2], F32,
                        kind="ExternalOutput")
    oe = nc.dram_tensor("oe", [2, P, N_Q, N_BJ, 2], U32,
                        kind="ExternalOutput")

    with tile.TileContext(nc) as tc:
        with (
            tc.tile_pool(name="singles", bufs=1) as singles,
            tc.tile_pool(name="hp", bufs=hbufs) as hp,
            tc.tile_pool(name="lp", bufs=lbufs) as lp,
            tc.tile_pool(name="big", bufs=1) as big,
            tc.tile_pool(name="ep", bufs=2) as ep,
            tc.tile_pool(name="psl", bufs=2, space="PSUM") as psl,
            tc.tile_pool(name="psx", bufs=2, space="PSUM") as psx,
            tc.tile_pool(name="pst", bufs=2, space="PSUM") as pst,
        ):
            gt = singles.tile([P, NCH, 2 * E], BF16)
            nc.sync.dma_start(out=gt, in_=g2[:])
            btile = singles.tile([P, N_BLK, E], F32)
            nc.sync.dma_start(out=btile, in_=bt[:])
            mabt = singles.tile([P, 2, E], F32)
            nc.sync.dma_start(out=mabt, in_=mm[:])

            def half_body(half):
                psA = psl.tile([P, 512], F32, tag="psA")   # A | X1*256
                psX = psx.tile([P, 512], F32, tag="psX")   # X2*256
                for ld in range(NCH // CQL):
                    lt = lp.tile([P, CQL, T_HALF], F8, tag="lt")
                    # ACT HWDGE ring: fp8 loads on a separate FIFO from the
                    # bf16 loads so the two input streams interleave freely
                    nc.scalar.dma_start(
                        out=lt, in_=hl[half, :, ld * CQL:(ld + 1) * CQL, :])
                    for j in range(CQL):
                        c = ld * CQL + j
                        for q in range(N_Q):
                            nc.tensor.matmul(
                                psX[32 * q:32 * q + E, :],
                                lhsT=gt[:, c, 0:E],
                                rhs=lt[:, j, q * 512:(q + 1) * 512],
                                start=(c == 0), stop=(c == NCH - 1),
                                tile_position=(0, 32 * q),
                                skip_group_check=True)
                for ld in range(NCH // CQH):
                    ht = hp.tile([P, CQH, T_HALF], BF16, tag="ht")
                    nc.sync.dma_start(
                        out=ht, in_=hh[half, :, ld * CQH:(ld + 1) * CQH, :])
                    for j in range(CQH):
                        c = ld * CQH + j
                        for q in range(N_Q):
                            nc.tensor.matmul(
                                psA[32 * q:32 * q + 2 * E, :],
                                lhsT=gt[:, c, :],
                                rhs=ht[:, j, q * 512:(q + 1) * 512],
                                start=(c == 0), stop=(c == NCH - 1),
                                tile_position=(0, 32 * q),
                                skip_group_check=True)

                # PSUM -> SBUF moves (PE matmuls read SBUF only; full-bank
                # reads give RAW deps on every col-group's accumulation)
                a16 = big.tile([P, 512], F32, tag="a16")
                nc.vector.tensor_copy(out=a16, in_=psA)
                x2s = big.tile([P, 512], F32, tag="x2s")
                nc.vector.tensor_copy(out=x2s, in_=psX)

                # combine-transpose: per block b=(q,bj), tokens {4k+bj}:
                #   tp[:, b] = a16[32q:32q+16]^T @ [I; 2^-8 I]
                #            + x2s[32q:32q+8]^T @ (2^-8 I)
                tp = pst.tile([P, 512], F32, tag="tp")
                for q in range(N_Q):
                    slA = slice(32 * q, 32 * q + 2 * E)
                    slX = slice(32 * q, 32 * q + E)
                    aR = a16[slA, :].rearrange("e (k bj) -> e bj k", bj=N_BJ)
                    xR = x2s[slX, :].rearrange("e (k bj) -> e bj k", bj=N_BJ)
                    for bj in range(N_BJ):
                        b = q * N_BJ + bj
                        nc.tensor.matmul(
                            tp[:, b * E:(b + 1) * E], lhsT=aR[:, bj, :],
                            rhs=mabt[slA, 0, :], start=True, stop=False,
                            tile_position=(32 * q, 0),
                            skip_group_check=True)
                        nc.tensor.matmul(
                            tp[:, b * E:(b + 1) * E], lhsT=xR[:, bj, :],
                            rhs=mabt[slX, 1, :], start=False, stop=True,
                            tile_position=(32 * q, 0),
                            skip_group_check=True)

                # sc = tp + bias (token-major; bias varies along free dim)
                sc = ep.tile([P, N_BLK, E], F32, tag="sc")
                nc.vector.tensor_tensor(
                    out=sc, in0=tp[:, 0:N_BLK * E].rearrange(
                        "p (b e) -> p b e", e=E),
                    in1=btile, op=mybir.AluOpType.add)

                mx = ep.tile([P, N_BLK, E], F32, tag="mx")
                mi = ep.tile([P, N_BLK, E], U32, tag="mi")
                for b in range(N_BLK):
                    nc.vector.max(out=mx[:, b, :], in_=sc[:, b, :])
                for b in range(N_BLK):
                    nc.vector.max_index(out=mi[:, b, :],
                                        in_max=mx[:, b, :],
                                        in_values=sc[:, b, :])

                dt_ = ep.tile([P, N_BLK], F32, tag="dt")
                nc.vector.tensor_tensor(
                    out=dt_, in0=mx[:, :, 1], in1=mx[:, :, 0],
                    op=mybir.AluOpType.subtract)
                et = ep.tile([P, N_BLK], F32, tag="et")
                nc.scalar.activation(
                    out=et, in_=dt_, func=mybir.ActivationFunctionType.Exp)
                st = ep.tile([P, N_BLK], F32, tag="st")
                nc.vector.tensor_scalar_add(st, et, 1.0)
                rt = ep.tile([P, N_BLK], F32, tag="rt")
                nc.vector.reciprocal(out=rt, in_=st)

                owt = ep.tile([P, N_BLK, 2], F32, tag="owt")
                nc.vector.tensor_copy(out=owt[:, :, 0], in_=rt)
                nc.vector.tensor_tensor(
                    out=owt[:, :, 1], in0=et, in1=rt,
                    op=mybir.AluOpType.mult)

                # ACT HWDGE ring: output stores off the SP ring so the
                # next half's input loads don't queue behind them
                nc.scalar.dma_start(
                    out=ow[half], in_=owt.rearrange(
                        "k (q bj) u -> k q bj u", q=N_Q))
                nc.scalar.dma_start(
                    out=oe[half], in_=mi[:, :, 0:2].rearrange(
                        "k (q bj) u -> k q bj u", q=N_Q))

            def body():
                for half in range(2):
                    half_body(half)

            if n_rep == 1:
                body()
            else:
                with tc.For_i(0, n_rep, 1):
                    body()

    nc.finalize()
    return nc


def _get_nc():
    if "nc" not in _NC_CACHE:
        _NC_CACHE["nc"] = build_router_nc()
    return _NC_CACHE["nc"]


def make_gate_inputs(pressure_bias, temperature_field, gate_w):
    gw = np.asarray(gate_w, dtype=np.float32)
    pb = np.asarray(pressure_bias, np.float32)
    temp = np.asarray(temperature_field, np.float32)
    it = 1.0 / np.clip(temp, np.float32(0.1), np.float32(10.0))
    gs = gw * it[:, None]
    gT = np.ascontiguousarray(gs.T)                         # [D, E]
    gh = gT.astype(ml_dtypes.bfloat16)
    gl = ((gT - gh.astype(np.float32)) * LO_SCALE).astype(ml_dtypes.bfloat16)
    gcomb = np.concatenate([gh, gl], axis=1)                # [D, 16]
    g2 = np.ascontiguousarray(
        gcomb.reshape(NCH, P, 2 * E).transpose(1, 0, 2))    # [P, NCH, 16]
    eye = np.eye(E, dtype=np.float32)
    mm = np.zeros((P, 2, E), np.float32)
    for q in range(N_Q):
        mm[32 * q:32 * q + E, 0, :] = eye
        mm[32 * q + E:32 * q + 2 * E, 0, :] = eye * INV_LO
        mm[32 * q:32 * q + E, 1, :] = eye * INV_LO
    bias = (pb * it).astype(np.float32)
    bt = np.ascontiguousarray(np.broadcast_to(bias, (P, N_BLK, E)))
    return g2, mm, bt


def make_h_inputs(hs_core):
    hT = np.ascontiguousarray(hs_core.T)
    hh_f = hT.astype(ml_dtypes.bfloat16)
    r = (hT - hh_f.astype(np.float32)) * np.float32(LO_SCALE)
    hl_f = r.astype(ml_dtypes.float8_e4m3)
    hh_dev = np.ascontiguousarray(
        hh_f.reshape(NCH, P, 2, T_HALF).transpose(2, 1, 0, 3))
    hl_dev = np.ascontiguousarray(
        hl_f.reshape(NCH, P, 2, T_HALF).transpose(2, 1, 0, 3))
    return hh_dev, hl_dev


def unshuffle_out(arr, t_core):
    return np.ascontiguousarray(
        arr.transpose(0, 2, 1, 3, 4).reshape(t_core, arr.shape[-1]))


def kernel(hidden_states, pressure_bias, temperature_field, gate_w):
    hs = np.ascontiguousarray(np.asarray(hidden_states, dtype=np.float32))
    hs = hs.reshape(T_TOTAL, D)
    g2, mm, bt = make_gate_inputs(pressure_bias, temperature_field, gate_w)

    in_maps = []
    for i in range(N_CORES):
        hh_dev, hl_dev = make_h_inputs(hs[i * T_CORE:(i + 1) * T_CORE])
        in_maps.append({"hh": hh_dev, "hl": hl_dev, "g2": g2,
                        "mm": mm, "bt": bt})

    nc = _get_nc()
    global LAST_RESULT
    res = run_bass_kernel_spmd(nc, in_maps, core_ids=list(range(N_CORES)),
                               trace=TRACE)
    LAST_RESULT = res

    weights = np.empty((T_TOTAL, 2), np.float32)
    experts = np.empty((T_TOTAL, 2), np.int32)
    for i, r in enumerate(res.results):
        weights[i * T_CORE:(i + 1) * T_CORE] = unshuffle_out(r["ow"], T_CORE)
        experts[i * T_CORE:(i + 1) * T_CORE] = (
            unshuffle_out(r["oe"], T_CORE).astype(np.int32))

    return weights.reshape(B, S, 2), experts.reshape(B, S, 2)



# revision 2
# speedup vs baseline: 1.4343x; 1.4343x over previous
"""v6: fp16-only hidden stream (2 B/elt instead of 3).

  * h shipped as fp16 (32 MiB/core vs 48 MiB for v5's bf16+fp8 split) --
    the kernel is DMA-bound so traffic is the metric that matters.
  * Gate keeps ~f32 precision as an fp16 hi/lo pair folded into one
    16-wide stationary [gh | gl*2^11]; a single combine matmul per
    token-block contracts the 16 rows with M = [I; 2^-11 I] while
    transposing expert-major -> token-major.
  * h loads alternate between the SP and ACT HWDGE rings; the last
    half's load schedule tapers (8,8,8,4,2,1,1 chunks) so the final
    PE+DVE tail only waits on a 512 KiB transfer.
"""

import numpy as np

import concourse.bass as bass
import concourse.tile as tile
from concourse import bacc, mybir
from concourse.bass_utils import run_bass_kernel_spmd

F32 = mybir.dt.float32
F16 = mybir.dt.float16
U32 = mybir.dt.uint32

N_CORES = 8
B, S, D, E = 4, 8192, 4096, 8
T_TOTAL = B * S
T_CORE = T_TOTAL // N_CORES
P = 128
NCH = D // P                       # 32
T_HALF = T_CORE // 2               # 2048
N_Q = T_HALF // 512                # 4
N_BJ = 4
N_BLK = N_Q * N_BJ                 # 16
SC = 2048.0
INV_SC = 1.0 / SC
GMAX = 8                           # h-tile slot width (chunks)
# chunk-group load schedule per half; the last half tapers so the
# end-of-stream PE/DVE tail hides behind only a tiny transfer
GROUPS = {0: (8, 8, 8, 8), 1: (8, 8, 8, 4, 2, 1, 1)}

_NC_CACHE = {}

TRACE = False
LAST_RESULT = None


def build_router_nc(n_rep=1, hbufs=4):
    nc = bacc.Bacc(None, target_bir_lowering=False)

    hh = nc.dram_tensor("hh", [2, P, NCH, T_HALF], F16, kind="ExternalInput")
    g2 = nc.dram_tensor("g2", [P, NCH, 2 * E], F16, kind="ExternalInput")
    # combine matrix, host-prebuilt: at each 32q base, rows 0:8 = I and
    # rows 8:16 = 2^-11 I  (contracts [A | X1*2^11] -> A + X1)
    mm = nc.dram_tensor("mm", [P, E], F32, kind="ExternalInput")
    bt = nc.dram_tensor("bt", [P, N_BLK, E], F32, kind="ExternalInput")
    ow = nc.dram_tensor("ow", [2, P, N_Q, N_BJ, 2], F32,
                        kind="ExternalOutput")
    oe = nc.dram_tensor("oe", [2, P, N_Q, N_BJ, 2], U32,
                        kind="ExternalOutput")

    with tile.TileContext(nc) as tc:
        with (
            tc.tile_pool(name="singles", bufs=1) as singles,
            tc.tile_pool(name="hp", bufs=hbufs) as hp,
            tc.tile_pool(name="big", bufs=2) as big,
            tc.tile_pool(name="ep", bufs=2) as ep,
            tc.tile_pool(name="psl", bufs=2, space="PSUM") as psl,
            tc.tile_pool(name="pst", bufs=2, space="PSUM") as pst,
        ):
            gt = singles.tile([P, NCH, 2 * E], F16)
            nc.sync.dma_start(out=gt, in_=g2[:])
            btile = singles.tile([P, N_BLK, E], F32)
            nc.sync.dma_start(out=btile, in_=bt[:])
            mabt = singles.tile([P, E], F32)
            nc.sync.dma_start(out=mabt, in_=mm[:])

            qctr = [0]

            def half_body(half):
                psA = psl.tile([P, 512], F32, tag="psA")
                c0 = 0
                for g in GROUPS[half]:
                    ht = hp.tile([P, GMAX, T_HALF], F16, tag="ht")
                    eng = nc.sync if qctr[0] % 2 == 0 else nc.scalar
                    qctr[0] += 1
                    eng.dma_start(out=ht[:, 0:g, :],
                                  in_=hh[half, :, c0:c0 + g, :])
                    for j in range(g):
                        c = c0 + j
                        for q in range(N_Q):
                            nc.tensor.matmul(
                                psA[32 * q:32 * q + 2 * E, :],
                                lhsT=gt[:, c, :],
                                rhs=ht[:, j, q * 512:(q + 1) * 512],
                                start=(c == 0), stop=(c == NCH - 1),
                                tile_position=(0, 32 * q),
                                skip_group_check=True)
                    c0 += g

                # PSUM -> SBUF (PE matmuls read SBUF only)
                a16 = big.tile([P, 512], F32, tag="a16")
                nc.vector.tensor_copy(out=a16, in_=psA)

                # combine-transpose: per block b=(q,bj), tokens {4k+bj}:
                #   tp[:, b] = a16[32q:32q+16]^T @ [I; 2^-11 I]
                tp = pst.tile([P, N_BLK * E], F32, tag="tp")
                for q in range(N_Q):
                    slA = slice(32 * q, 32 * q + 2 * E)
                    aR = a16[slA, :].rearrange("e (k bj) -> e bj k", bj=N_BJ)
                    for bj in range(N_BJ):
                        b = q * N_BJ + bj
                        nc.tensor.matmul(
                            tp[:, b * E:(b + 1) * E], lhsT=aR[:, bj, :],
                            rhs=mabt[slA, :], start=True, stop=True,
                            tile_position=(32 * q, 0),
                            skip_group_check=True)

                # sc = tp + bias (token-major; bias varies along free dim)
                sc = ep.tile([P, N_BLK, E], F32, tag="sc")
                nc.vector.tensor_tensor(
                    out=sc, in0=tp[:, 0:N_BLK * E].rearrange(
                        "p (b e) -> p b e", e=E),
                    in1=btile, op=mybir.AluOpType.add)

                mx = ep.tile([P, N_BLK, E], F32, tag="mx")
                mi = ep.tile([P, N_BLK, E], U32, tag="mi")
                for b in range(N_BLK):
                    nc.vector.max(out=mx[:, b, :], in_=sc[:, b, :])
                for b in range(N_BLK):
                    nc.vector.max_index(out=mi[:, b, :],
                                        in_max=mx[:, b, :],
                                        in_values=sc[:, b, :])

                dt_ = ep.tile([P, N_BLK], F32, tag="dt")
                nc.vector.tensor_tensor(
                    out=dt_, in0=mx[:, :, 1], in1=mx[:, :, 0],
                    op=mybir.AluOpType.subtract)
                et = ep.tile([P, N_BLK], F32, tag="et")
                nc.scalar.activation(
                    out=et, in_=dt_, func=mybir.ActivationFunctionType.Exp)
                st = ep.tile([P, N_BLK], F32, tag="st")
                nc.vector.tensor_scalar_add(st, et, 1.0)
                rt = ep.tile([P, N_BLK], F32, tag="rt")
                nc.vector.reciprocal(out=rt, in_=st)

                owt = ep.tile([P, N_BLK, 2], F32, tag="owt")
                nc.vector.tensor_copy(out=owt[:, :, 0], in_=rt)
                nc.vector.tensor_tensor(
                    out=owt[:, :, 1], in0=et, in1=rt,
                    op=mybir.AluOpType.mult)

                nc.scalar.dma_start(
                    out=ow[half], in_=owt.rearrange(
                        "k (q bj) u -> k q bj u", q=N_Q))
                nc.scalar.dma_start(
                    out=oe[half], in_=mi[:, :, 0:2].rearrange(
                        "k (q bj) u -> k q bj u", q=N_Q))

            def body():
                for half in range(2):
                    half_body(half)

            if n_rep == 1:
                body()
            else:
                with tc.For_i(0, n_rep, 1):
                    body()

    nc.finalize()
    return nc


def _get_nc():
    if "nc" not in _NC_CACHE:
        _NC_CACHE["nc"] = build_router_nc()
    return _NC_CACHE["nc"]


def make_gate_inputs(pressure_bias, temperature_field, gate_w):
    gw = np.asarray(gate_w, dtype=np.float32)
    pb = np.asarray(pressure_bias, np.float32)
    temp = np.asarray(temperature_field, np.float32)
    it = 1.0 / np.clip(temp, np.float32(0.1), np.float32(10.0))
    gs = gw * it[:, None]
    gT = np.ascontiguousarray(gs.T)                         # [D, E]
    gh = gT.astype(np.float16)
    gl = ((gT - gh.astype(np.float32)) * SC).astype(np.float16)
    gcomb = np.concatenate([gh, gl], axis=1)                # [D, 16]
    g2 = np.ascontiguousarray(
        gcomb.reshape(NCH, P, 2 * E).transpose(1, 0, 2))    # [P, NCH, 16]
    eye = np.eye(E, dtype=np.float32)
    mm = np.zeros((P, E), np.float32)
    for q in range(N_Q):
        mm[32 * q:32 * q + E, :] = eye
        mm[32 * q + E:32 * q + 2 * E, :] = eye * INV_SC
    bias = (pb * it).astype(np.float32)
    bt = np.ascontiguousarray(np.broadcast_to(bias, (P, N_BLK, E)))
    return g2, mm, bt


def make_h_inputs(hs_core):
    hT = np.ascontiguousarray(hs_core.T).astype(np.float16)
    return np.ascontiguousarray(
        hT.reshape(NCH, P, 2, T_HALF).transpose(2, 1, 0, 3))


def unshuffle_out(arr, t_core):
    return np.ascontiguousarray(
        arr.transpose(0, 2, 1, 3, 4).reshape(t_core, arr.shape[-1]))


def kernel(hidden_states, pressure_bias, temperature_field, gate_w):
    hs = np.ascontiguousarray(np.asarray(hidden_states, dtype=np.float32))
    hs = hs.reshape(T_TOTAL, D)
    g2, mm, bt = make_gate_inputs(pressure_bias, temperature_field, gate_w)

    in_maps = []
    for i in range(N_CORES):
        hh_dev = make_h_inputs(hs[i * T_CORE:(i + 1) * T_CORE])
        in_maps.append({"hh": hh_dev, "g2": g2, "mm": mm, "bt": bt})

    nc = _get_nc()
    global LAST_RESULT
    res = run_bass_kernel_spmd(nc, in_maps, core_ids=list(range(N_CORES)),
                               trace=TRACE)
    LAST_RESULT = res

    weights = np.empty((T_TOTAL, 2), np.float32)
    experts = np.empty((T_TOTAL, 2), np.int32)
    for i, r in enumerate(res.results):
        weights[i * T_CORE:(i + 1) * T_CORE] = unshuffle_out(r["ow"], T_CORE)
        experts[i * T_CORE:(i + 1) * T_CORE] = (
            unshuffle_out(r["oe"], T_CORE).astype(np.int32))

    return weights.reshape(B, S, 2), experts.reshape(B, S, 2)


# revision 11
# speedup vs baseline: 1.5735x; 1.0971x over previous
"""v8: fp16-only hidden stream, single in-order DMA ring, 3 segments.

  * h shipped as fp16 (32 MiB/core vs 48 MiB for v5's bf16+fp8 split) --
    the kernel is DMA-bound so traffic is the metric that matters.
  * Gate keeps ~f32 precision as an fp16 hi/lo pair folded into one
    16-wide stationary [gh | gl*2^11]; a single combine matmul per
    token-block contracts the 16 rows with M = [I; 2^-11 I] while
    transposing expert-major -> token-major.
  * All h loads ride ONE HWDGE ring (SP) so chunks arrive in exactly
    the order PE consumes them -- a second ring reorders arrivals and
    stalls the in-order PE stream.
  * Tokens split 2048/1536/512: the big segments' epilogues hide under
    later DMA traffic; only the 512-token segment's (4-block) epilogue
    is exposed after the last byte, and its load schedule tapers to
    single 256 KiB chunks.
"""

import numpy as np

import concourse.bass as bass
import concourse.tile as tile
from concourse import bacc, mybir
from concourse.bass_utils import run_bass_kernel_spmd

F32 = mybir.dt.float32
F16 = mybir.dt.float16
U32 = mybir.dt.uint32

N_CORES = 8
B, S, D, E = 4, 8192, 4096, 8
T_TOTAL = B * S
T_CORE = T_TOTAL // N_CORES        # 4096
P = 128
NCH = D // P                       # 32
N_BJ = 4
NG = T_CORE // 512                 # 8 col-groups of 512 tokens
SC = 2048.0
INV_SC = 1.0 / SC

# (token offset, token count, group schedule): start taper gets PE fed
# early; end taper keeps the final exposed transfer small
SEGS = (
    (0, 2048, (1, 1, 2, 4, 8, 8, 8)),
    (2048, 1536, (8, 8, 8, 8)),
    (3584, 512, (8, 8, 8, 4, 2, 1, 1)),
)

_NC_CACHE = {}

TRACE = False
LAST_RESULT = None


def build_router_nc(n_rep=1, hbufs=5):
    nc = bacc.Bacc(None, target_bir_lowering=False)

    hh = nc.dram_tensor("hh", [P, NCH, T_CORE], F16, kind="ExternalInput")
    g2 = nc.dram_tensor("g2", [P, NCH, 2 * E], F16, kind="ExternalInput")
    # combine matrix, host-prebuilt: at each 32q base, rows 0:8 = I and
    # rows 8:16 = 2^-11 I  (contracts [A | X1*2^11] -> A + X1)
    mm = nc.dram_tensor("mm", [P, E], F32, kind="ExternalInput")
    bt = nc.dram_tensor("bt", [P, NG * N_BJ, E], F32, kind="ExternalInput")
    ow = nc.dram_tensor("ow", [P, NG, N_BJ, 2], F32, kind="ExternalOutput")
    oe = nc.dram_tensor("oe", [P, NG, N_BJ, 2], U32, kind="ExternalOutput")

    with tile.TileContext(nc) as tc:
        with (
            tc.tile_pool(name="singles", bufs=1) as singles,
            tc.tile_pool(name="hp", bufs=hbufs) as hp,
            tc.tile_pool(name="big", bufs=2) as big,
            tc.tile_pool(name="ep", bufs=2) as ep,
            tc.tile_pool(name="psl", bufs=2, space="PSUM") as psl,
            tc.tile_pool(name="pst", bufs=2, space="PSUM") as pst,
        ):
            # gt gates the first matmul -> first on the h (SP) ring;
            # epilogue-only singles ride the Pool SWDGE ring
            gt = singles.tile([P, NCH, 2 * E], F16)
            nc.sync.dma_start(out=gt, in_=g2[:])
            btile = singles.tile([P, NG * N_BJ, E], F32)
            nc.gpsimd.dma_start(out=btile, in_=bt[:])
            mabt = singles.tile([P, E], F32)
            nc.gpsimd.dma_start(out=mabt, in_=mm[:])

            def seg_body(t0, tn, groups):
                nq = tn // 512
                qg0 = t0 // 512
                psA = psl.tile([P, 512], F32, tag="psA")
                c0 = 0
                for g in groups:
                    ht = hp.tile([P, 8, 2048], F16, tag="ht")
                    nc.sync.dma_start(out=ht[:, 0:g, 0:tn],
                                      in_=hh[:, c0:c0 + g, t0:t0 + tn])
                    for j in range(g):
                        c = c0 + j
                        for q in range(nq):
                            nc.tensor.matmul(
                                psA[32 * q:32 * q + 2 * E, :],
                                lhsT=gt[:, c, :],
                                rhs=ht[:, j, q * 512:(q + 1) * 512],
                                start=(c == 0), stop=(c == NCH - 1),
                                tile_position=(0, 32 * q),
                                skip_group_check=True)
                    c0 += g

                # PSUM -> SBUF (PE matmuls read SBUF only); valid rows
                # are the col-group bands 32q..32q+16
                hi = 32 * (nq - 1) + 2 * E
                a16 = big.tile([P, 512], F32, tag="a16")
                nc.vector.tensor_copy(out=a16[0:hi, :], in_=psA[0:hi, :])

                # combine-transpose: per block b=(q,bj), tokens {4k+bj}:
                #   tp[:, b] = a16[32q:32q+16]^T @ [I; 2^-11 I]
                nb = nq * N_BJ
                tp = pst.tile([P, NG * N_BJ * E], F32, tag="tp")
                for q in range(nq):
                    slA = slice(32 * q, 32 * q + 2 * E)
                    aR = a16[slA, :].rearrange("e (k bj) -> e bj k", bj=N_BJ)
                    for bj in range(N_BJ):
                        b = q * N_BJ + bj
                        nc.tensor.matmul(
                            tp[:, b * E:(b + 1) * E], lhsT=aR[:, bj, :],
                            rhs=mabt[slA, :], start=True, stop=True,
                            tile_position=(32 * q, 0),
                            skip_group_check=True)

                # sc = tp + bias (token-major; bias varies along free dim)
                sc = ep.tile([P, NG * N_BJ, E], F32, tag="sc")
                nc.vector.tensor_tensor(
                    out=sc[:, 0:nb, :], in0=tp[:, 0:nb * E].rearrange(
                        "p (b e) -> p b e", e=E),
                    in1=btile[:, 0:nb, :],
                    op=mybir.AluOpType.add)

                mx = ep.tile([P, NG * N_BJ, E], F32, tag="mx")
                mi = ep.tile([P, NG * N_BJ, E], U32, tag="mi")
                for b in range(nb):
                    nc.vector.max(out=mx[:, b, :], in_=sc[:, b, :])
                for b in range(nb):
                    nc.vector.max_index(out=mi[:, b, :],
                                        in_max=mx[:, b, :],
                                        in_values=sc[:, b, :])
                # indices are ready before the softmax chain: ship them
                # first so the final ow DMA is the only post-chain cost
                nc.scalar.dma_start(
                    out=oe[:, qg0:qg0 + nq], in_=mi[:, 0:nb, 0:2].rearrange(
                        "k (q bj) u -> k q bj u", q=nq))

                dt_ = ep.tile([P, NG * N_BJ], F32, tag="dt")
                nc.vector.tensor_tensor(
                    out=dt_[:, 0:nb], in0=mx[:, 0:nb, 1], in1=mx[:, 0:nb, 0],
                    op=mybir.AluOpType.subtract)
                et = ep.tile([P, NG * N_BJ], F32, tag="et")
                nc.scalar.activation(
                    out=et[:, 0:nb], in_=dt_[:, 0:nb],
                    func=mybir.ActivationFunctionType.Exp)
                st = ep.tile([P, NG * N_BJ], F32, tag="st")
                nc.vector.tensor_scalar_add(st[:, 0:nb], et[:, 0:nb], 1.0)
                rt = ep.tile([P, NG * N_BJ], F32, tag="rt")
                nc.vector.reciprocal(out=rt[:, 0:nb], in_=st[:, 0:nb])

                owt = ep.tile([P, NG * N_BJ, 2], F32, tag="owt")
                nc.vector.tensor_copy(out=owt[:, 0:nb, 0], in_=rt[:, 0:nb])
                nc.vector.tensor_tensor(
                    out=owt[:, 0:nb, 1], in0=et[:, 0:nb], in1=rt[:, 0:nb],
                    op=mybir.AluOpType.mult)

                nc.scalar.dma_start(
                    out=ow[:, qg0:qg0 + nq], in_=owt[:, 0:nb].rearrange(
                        "k (q bj) u -> k q bj u", q=nq))

            def body():
                for t0, tn, groups in SEGS:
                    seg_body(t0, tn, groups)

            if n_rep == 1:
                body()
            else:
                with tc.For_i(0, n_rep, 1):
                    body()

    nc.finalize()
    return nc


def _get_nc():
    if "nc" not in _NC_CACHE:
        _NC_CACHE["nc"] = build_router_nc()
    return _NC_CACHE["nc"]


def make_gate_inputs(pressure_bias, temperature_field, gate_w):
    gw = np.asarray(gate_w, dtype=np.float32)
    pb = np.asarray(pressure_bias, np.float32)
    temp = np.asarray(temperature_field, np.float32)
    it = 1.0 / np.clip(temp, np.float32(0.1), np.float32(10.0))
    gs = gw * it[:, None]
    gT = np.ascontiguousarray(gs.T)                         # [D, E]
    gh = gT.astype(np.float16)
    gl = ((gT - gh.astype(np.float32)) * SC).astype(np.float16)
    gcomb = np.concatenate([gh, gl], axis=1)                # [D, 16]
    g2 = np.ascontiguousarray(
        gcomb.reshape(NCH, P, 2 * E).transpose(1, 0, 2))    # [P, NCH, 16]
    eye = np.eye(E, dtype=np.float32)
    mm = np.zeros((P, E), np.float32)
    for q in range(4):
        mm[32 * q:32 * q + E, :] = eye
        mm[32 * q + E:32 * q + 2 * E, :] = eye * INV_SC
    bias = (pb * it).astype(np.float32)
    bt = np.ascontiguousarray(np.broadcast_to(bias, (P, NG * N_BJ, E)))
    return g2, mm, bt


def make_h_inputs(hs_core):
    hT = np.ascontiguousarray(hs_core.T).astype(np.float16)
    return np.ascontiguousarray(
        hT.reshape(NCH, P, T_CORE).transpose(1, 0, 2))


def unshuffle_out(arr, t_core):
    # arr [P, NG, N_BJ, u]: token = 512*g + 4*k + bj
    return np.ascontiguousarray(
        arr.transpose(1, 0, 2, 3).reshape(t_core, arr.shape[-1]))


def kernel(hidden_states, pressure_bias, temperature_field, gate_w):
    hs = np.ascontiguousarray(np.asarray(hidden_states, dtype=np.float32))
    hs = hs.reshape(T_TOTAL, D)
    g2, mm, bt = make_gate_inputs(pressure_bias, temperature_field, gate_w)

    in_maps = []
    for i in range(N_CORES):
        hh_dev = make_h_inputs(hs[i * T_CORE:(i + 1) * T_CORE])
        in_maps.append({"hh": hh_dev, "g2": g2, "mm": mm, "bt": bt})

    nc = _get_nc()
    global LAST_RESULT
    res = run_bass_kernel_spmd(nc, in_maps, core_ids=list(range(N_CORES)),
                               trace=TRACE)
    LAST_RESULT = res

    weights = np.empty((T_TOTAL, 2), np.float32)
    experts = np.empty((T_TOTAL, 2), np.int32)
    for i, r in enumerate(res.results):
        weights[i * T_CORE:(i + 1) * T_CORE] = unshuffle_out(r["ow"], T_CORE)
        experts[i * T_CORE:(i + 1) * T_CORE] = (
            unshuffle_out(r["oe"], T_CORE).astype(np.int32))

    return weights.reshape(B, S, 2), experts.reshape(B, S, 2)


# revision 23
# speedup vs baseline: 1.6047x; 1.0198x over previous
"""v8: fp16-only hidden stream, single in-order DMA ring, 3 segments.

  * h shipped as fp16 (32 MiB/core vs 48 MiB for v5's bf16+fp8 split) --
    the kernel is DMA-bound so traffic is the metric that matters.
  * Gate keeps ~f32 precision as an fp16 hi/lo pair folded into one
    16-wide stationary [gh | gl*2^11]; a single combine matmul per
    token-block contracts the 16 rows with M = [I; 2^-11 I] while
    transposing expert-major -> token-major.
  * All h loads ride ONE HWDGE ring (SP) so chunks arrive in exactly
    the order PE consumes them -- a second ring reorders arrivals and
    stalls the in-order PE stream.
  * Tokens split 2048/1536/512: the big segments' epilogues hide under
    later DMA traffic; only the 512-token segment's (4-block) epilogue
    is exposed after the last byte, and its load schedule tapers to
    single 256 KiB chunks.
"""

import numpy as np

import concourse.bass as bass
import concourse.tile as tile
from concourse import bacc, mybir
from concourse.bass_utils import run_bass_kernel_spmd

F32 = mybir.dt.float32
F16 = mybir.dt.float16
U32 = mybir.dt.uint32

N_CORES = 8
B, S, D, E = 4, 8192, 4096, 8
T_TOTAL = B * S
T_CORE = T_TOTAL // N_CORES        # 4096
P = 128
NCH = D // P                       # 32
N_BJ = 4
NG = T_CORE // 512                 # 8 col-groups of 512 tokens
SC = 2048.0
INV_SC = 1.0 / SC

# (token offset, token count, group schedule): start taper gets PE fed
# early; end taper keeps the final exposed transfer small
SEGS = (
    (0, 2048, (1, 1, 2, 4, 4, 5, 5, 5, 5)),
    (2048, 1536, (5, 5, 5, 5, 5, 5, 2)),
    (3584, 512, (5, 5, 5, 5, 5, 3, 2, 1, 1)),
)

# junk-matmul padding (into a scratch PSUM band), in ns of PE time.
# PE's DVFS ramp resets on any idle: an idle PE restarts at 0.65/1.2
# GHz and only reaches 2.4 GHz after 3us of continuous execution.  The
# pads keep PE exactly busy between chunk arrivals and across the
# segment-boundary PSUM-copy latency so every real matmul runs at full
# clock and PE carries ~zero backlog into the exposed tail.  Tuned
# greedily against TimelineSim arrival times.
PADS = {"chunk": [0] * 96, "boundary": [0] * 3}

_NC_CACHE = {}

TRACE = False
LAST_RESULT = None


def build_router_nc(n_rep=1, hbufs=8, pads=None):
    nc = bacc.Bacc(None, target_bir_lowering=False)

    hh = nc.dram_tensor("hh", [P, NCH, T_CORE], F16, kind="ExternalInput")
    g2 = nc.dram_tensor("g2", [P, NCH, 2 * E], F16, kind="ExternalInput")
    # combine matrix, host-prebuilt: at each 32q base, rows 0:8 = I and
    # rows 8:16 = 2^-11 I  (contracts [A | X1*2^11] -> A + X1)
    mm = nc.dram_tensor("mm", [P, E], F32, kind="ExternalInput")
    bt = nc.dram_tensor("bt", [P, NG * N_BJ, E], F32, kind="ExternalInput")
    ob = nc.dram_tensor("ob", [P, NG, N_BJ, 4], F32, kind="ExternalOutput")

    if pads is None:
        pads = PADS

    with tile.TileContext(nc) as tc:
        with (
            tc.tile_pool(name="singles", bufs=1) as singles,
            tc.tile_pool(name="hp", bufs=hbufs) as hp,
            tc.tile_pool(name="big", bufs=2) as big,
            tc.tile_pool(name="ep", bufs=2) as ep,
            tc.tile_pool(name="psl", bufs=2, space="PSUM") as psl,
            tc.tile_pool(name="pst", bufs=2, space="PSUM") as pst,
            tc.tile_pool(name="psj", bufs=1, space="PSUM") as psj,
        ):
            # h rides the SP ring alone (strictly in PE order); gt on
            # the ACT ring lands concurrently with h chunk 0; the
            # epilogue-only singles go on the Pool SWDGE ring
            gt = singles.tile([P, NCH, 2 * E], F16)
            nc.scalar.dma_start(out=gt, in_=g2[:])
            btile = singles.tile([P, NG * N_BJ, E], F32)
            mabt = singles.tile([P, E], F32)

            def load_singles():
                nc.gpsimd.dma_start(out=btile, in_=bt[:])
                nc.gpsimd.dma_start(out=mabt, in_=mm[:])

            psJ = psj.tile([P, 512], F32, tag="psJ")

            def emit_pad(cols_total, rhs_tile, j):
                # realize pad-columns of PE work as junk matmuls: full
                # 512-col units plus one fractional column-slice
                while cols_total >= 64:
                    cols = 512 if cols_total >= 512 else cols_total
                    nc.tensor.matmul(
                        psJ[0:2 * E, 0:cols], lhsT=gt[:, 0, :],
                        rhs=rhs_tile[:, j, 0:cols],
                        start=True, stop=True,
                        tile_position=(0, 0),
                        skip_group_check=True)
                    cols_total -= cols

            def seg_body(t0, tn, groups, si):
                nq = tn // 512
                qg0 = t0 // 512
                psA = psl.tile([P, 512], F32, tag="psA")
                c0 = 0
                for gi, g in enumerate(groups):
                    ht = hp.tile([P, 5, 2048], F16, tag="ht")
                    nc.sync.dma_start(out=ht[:, 0:g, 0:tn],
                                      in_=hh[:, c0:c0 + g, t0:t0 + tn])
                    if si == 0 and gi == 1:
                        load_singles()
                    for j in range(g):
                        c = c0 + j
                        for q in range(nq):
                            nc.tensor.matmul(
                                psA[32 * q:32 * q + 2 * E, :],
                                lhsT=gt[:, c, :],
                                rhs=ht[:, j, q * 512:(q + 1) * 512],
                                start=(c == 0), stop=(c == NCH - 1),
                                tile_position=(0, 32 * q),
                                skip_group_check=True)
                        emit_pad(pads["chunk"][si * NCH + c], ht, j)
                    c0 += g
                    last_ht = ht

                # PSUM -> SBUF (PE matmuls read SBUF only); valid rows
                # are the col-group bands 32q..32q+16
                hi = 32 * (nq - 1) + 2 * E
                a16 = big.tile([P, 512], F32, tag="a16")
                nc.vector.tensor_copy(out=a16[0:hi, :], in_=psA[0:hi, :])
                # bridge PE over the PSUM-copy latency (combine matmuls
                # wait on a16) so its DVFS run doesn't break here
                emit_pad(pads["boundary"][si], last_ht, 0)

                # combine-transpose: per block b=(q,bj), tokens {4k+bj}:
                #   tp[:, b] = a16[32q:32q+16]^T @ [I; 2^-11 I]
                nb = nq * N_BJ
                tp = pst.tile([P, NG * N_BJ * E], F32, tag="tp")
                for q in range(nq):
                    slA = slice(32 * q, 32 * q + 2 * E)
                    aR = a16[slA, :].rearrange("e (k bj) -> e bj k", bj=N_BJ)
                    for bj in range(N_BJ):
                        b = q * N_BJ + bj
                        nc.tensor.matmul(
                            tp[:, b * E:(b + 1) * E], lhsT=aR[:, bj, :],
                            rhs=mabt[slA, :], start=True, stop=True,
                            tile_position=(32 * q, 0),
                            skip_group_check=True)

                # sc = tp + bias (token-major; bias varies along free dim)
                sc = ep.tile([P, NG * N_BJ, E], F32, tag="sc")
                nc.vector.tensor_tensor(
                    out=sc[:, 0:nb, :], in0=tp[:, 0:nb * E].rearrange(
                        "p (b e) -> p b e", e=E),
                    in1=btile[:, 0:nb, :],
                    op=mybir.AluOpType.add)

                mx = ep.tile([P, NG * N_BJ, E], F32, tag="mx")
                mi = ep.tile([P, NG * N_BJ, E], U32, tag="mi")
                for b in range(nb):
                    nc.vector.max(out=mx[:, b, :], in_=sc[:, b, :])

                # top-2 softmax = sigmoid(+-(v0-v1)): one DVE subtract,
                # two ACT sigmoids; the ACT passes overlap the DVE index
                # pass below
                dt_ = ep.tile([P, NG * N_BJ], F32, tag="dt")
                nc.vector.tensor_tensor(
                    out=dt_[:, 0:nb], in0=mx[:, 0:nb, 0], in1=mx[:, 0:nb, 1],
                    op=mybir.AluOpType.subtract)
                ov = ep.tile([P, NG * N_BJ, 4], F32, tag="ov")
                nc.scalar.activation(
                    out=ov[:, 0:nb, 0], in_=dt_[:, 0:nb],
                    func=mybir.ActivationFunctionType.Sigmoid)
                nc.scalar.activation(
                    out=ov[:, 0:nb, 1], in_=dt_[:, 0:nb],
                    func=mybir.ActivationFunctionType.Sigmoid, scale=-1.0)
                for b in range(nb):
                    nc.vector.max_index(out=mi[:, b, :],
                                        in_max=mx[:, b, :],
                                        in_values=sc[:, b, :])
                nc.vector.tensor_copy(out=ov[:, 0:nb, 2:4],
                                      in_=mi[:, 0:nb, 0:2])

                # one merged [w1 w2 e1 e2] store; mid-stream segments go
                # out the ACT ring (their wait would stall the in-order
                # SP h queue), the final one uses the now-idle SP ring
                out_eng = nc.sync if si == len(SEGS) - 1 else nc.scalar
                out_eng.dma_start(
                    out=ob[:, qg0:qg0 + nq], in_=ov[:, 0:nb].rearrange(
                        "k (q bj) u -> k q bj u", q=nq))

            def body():
                for si, (t0, tn, groups) in enumerate(SEGS):
                    seg_body(t0, tn, groups, si)

            if n_rep == 1:
                body()
            else:
                with tc.For_i(0, n_rep, 1):
                    body()

    nc.finalize()
    return nc


def _get_nc():
    if "nc" not in _NC_CACHE:
        _NC_CACHE["nc"] = build_router_nc()
    return _NC_CACHE["nc"]


def make_gate_inputs(pressure_bias, temperature_field, gate_w):
    gw = np.asarray(gate_w, dtype=np.float32)
    pb = np.asarray(pressure_bias, np.float32)
    temp = np.asarray(temperature_field, np.float32)
    it = 1.0 / np.clip(temp, np.float32(0.1), np.float32(10.0))
    gs = gw * it[:, None]
    gT = np.ascontiguousarray(gs.T)                         # [D, E]
    gh = gT.astype(np.float16)
    gl = ((gT - gh.astype(np.float32)) * SC).astype(np.float16)
    gcomb = np.concatenate([gh, gl], axis=1)                # [D, 16]
    g2 = np.ascontiguousarray(
        gcomb.reshape(NCH, P, 2 * E).transpose(1, 0, 2))    # [P, NCH, 16]
    eye = np.eye(E, dtype=np.float32)
    mm = np.zeros((P, E), np.float32)
    for q in range(4):
        mm[32 * q:32 * q + E, :] = eye
        mm[32 * q + E:32 * q + 2 * E, :] = eye * INV_SC
    bias = (pb * it).astype(np.float32)
    bt = np.ascontiguousarray(np.broadcast_to(bias, (P, NG * N_BJ, E)))
    return g2, mm, bt


def make_h_inputs(hs_core):
    hT = np.ascontiguousarray(hs_core.T).astype(np.float16)
    return np.ascontiguousarray(
        hT.reshape(NCH, P, T_CORE).transpose(1, 0, 2))


def unshuffle_out(arr, t_core):
    # arr [P, NG, N_BJ, u]: token = 512*g + 4*k + bj
    return np.ascontiguousarray(
        arr.transpose(1, 0, 2, 3).reshape(t_core, arr.shape[-1]))


def decode_idx(e_f):
    # device writes indices via a u32->f32 tensor_copy; accept either
    # value-convert (0.0..7.0) or raw bitcast (denormal) semantics
    if e_f.size and np.abs(e_f).max() < 1e-6:
        return np.ascontiguousarray(e_f).view(np.uint32).astype(np.int32)
    return np.round(e_f).astype(np.int32)


def kernel(hidden_states, pressure_bias, temperature_field, gate_w):
    hs = np.ascontiguousarray(np.asarray(hidden_states, dtype=np.float32))
    hs = hs.reshape(T_TOTAL, D)
    g2, mm, bt = make_gate_inputs(pressure_bias, temperature_field, gate_w)

    in_maps = []
    for i in range(N_CORES):
        hh_dev = make_h_inputs(hs[i * T_CORE:(i + 1) * T_CORE])
        in_maps.append({"hh": hh_dev, "g2": g2, "mm": mm, "bt": bt})

    nc = _get_nc()
    global LAST_RESULT
    res = run_bass_kernel_spmd(nc, in_maps, core_ids=list(range(N_CORES)),
                               trace=TRACE)
    LAST_RESULT = res

    weights = np.empty((T_TOTAL, 2), np.float32)
    experts = np.empty((T_TOTAL, 2), np.int32)
    for i, r in enumerate(res.results):
        o = unshuffle_out(r["ob"], T_CORE)
        weights[i * T_CORE:(i + 1) * T_CORE] = o[:, 0:2]
        experts[i * T_CORE:(i + 1) * T_CORE] = decode_idx(o[:, 2:4])

    return weights.reshape(B, S, 2), experts.reshape(B, S, 2)


# revision 24
# speedup vs baseline: 1.6072x; 1.0016x over previous
"""v8: fp16-only hidden stream, single in-order DMA ring, 3 segments.

  * h shipped as fp16 (32 MiB/core vs 48 MiB for v5's bf16+fp8 split) --
    the kernel is DMA-bound so traffic is the metric that matters.
  * Gate keeps ~f32 precision as an fp16 hi/lo pair folded into one
    16-wide stationary [gh | gl*2^11]; a single combine matmul per
    token-block contracts the 16 rows with M = [I; 2^-11 I] while
    transposing expert-major -> token-major.
  * All h loads ride ONE HWDGE ring (SP) so chunks arrive in exactly
    the order PE consumes them -- a second ring reorders arrivals and
    stalls the in-order PE stream.
  * Tokens split 2048/1536/512: the big segments' epilogues hide under
    later DMA traffic; only the 512-token segment's (4-block) epilogue
    is exposed after the last byte, and its load schedule tapers to
    single 256 KiB chunks.
"""

import numpy as np

import concourse.bass as bass
import concourse.tile as tile
from concourse import bacc, mybir
from concourse.bass_utils import run_bass_kernel_spmd

F32 = mybir.dt.float32
F16 = mybir.dt.float16
U32 = mybir.dt.uint32

N_CORES = 8
B, S, D, E = 4, 8192, 4096, 8
T_TOTAL = B * S
T_CORE = T_TOTAL // N_CORES        # 4096
P = 128
NCH = D // P                       # 32
N_BJ = 4
NG = T_CORE // 512                 # 8 col-groups of 512 tokens
SC = 2048.0
INV_SC = 1.0 / SC

# (token offset, token count, group schedule): start taper gets PE fed
# early; end taper keeps the final exposed transfer small
SEGS = (
    (0, 2048, (1, 1, 2, 4, 4, 5, 5, 5, 5)),
    (2048, 1536, (5, 5, 5, 5, 5, 5, 2)),
    (3584, 512, (5, 5, 5, 5, 5, 3, 2, 1, 1)),
)

# junk-matmul padding (into a scratch PSUM band), in ns of PE time.
# PE's DVFS ramp resets on any idle: an idle PE restarts at 0.65/1.2
# GHz and only reaches 2.4 GHz after 3us of continuous execution.  The
# pads keep PE exactly busy between chunk arrivals and across the
# segment-boundary PSUM-copy latency so every real matmul runs at full
# clock and PE carries ~zero backlog into the exposed tail.  Tuned
# greedily against TimelineSim arrival times.
PADS = {"chunk": [0] * 96, "boundary": [0] * 3}

_NC_CACHE = {}

TRACE = False
LAST_RESULT = None


def build_router_nc(n_rep=1, hbufs=8, pads=None):
    nc = bacc.Bacc(None, target_bir_lowering=False)

    hh = nc.dram_tensor("hh", [P, NCH, T_CORE], F16, kind="ExternalInput")
    g2 = nc.dram_tensor("g2", [P, NCH, 2 * E], F16, kind="ExternalInput")
    # combine matrix, host-prebuilt: at each 32q base, rows 0:8 = I and
    # rows 8:16 = 2^-11 I  (contracts [A | X1*2^11] -> A + X1)
    mm = nc.dram_tensor("mm", [P, E], F32, kind="ExternalInput")
    bt = nc.dram_tensor("bt", [P, 1, E], F32, kind="ExternalInput")
    ob = nc.dram_tensor("ob", [P, NG, N_BJ, 4], F32, kind="ExternalOutput")

    if pads is None:
        pads = PADS

    with tile.TileContext(nc) as tc:
        with (
            tc.tile_pool(name="singles", bufs=1) as singles,
            tc.tile_pool(name="hp", bufs=hbufs) as hp,
            tc.tile_pool(name="big", bufs=2) as big,
            tc.tile_pool(name="ep", bufs=2) as ep,
            tc.tile_pool(name="psl", bufs=2, space="PSUM") as psl,
            tc.tile_pool(name="pst", bufs=2, space="PSUM") as pst,
            tc.tile_pool(name="psj", bufs=1, space="PSUM") as psj,
        ):
            # h rides the SP ring alone (strictly in PE order); gt on
            # the ACT ring lands concurrently with h chunk 0; the
            # epilogue-only singles go on the Pool SWDGE ring
            gt = singles.tile([P, NCH, 2 * E], F16)
            nc.scalar.dma_start(out=gt, in_=g2[:])
            btile = singles.tile([P, 1, E], F32)
            mabt = singles.tile([P, E], F32)

            def load_singles():
                nc.gpsimd.dma_start(out=btile, in_=bt[:])
                nc.gpsimd.dma_start(out=mabt, in_=mm[:])

            psJ = psj.tile([P, 512], F32, tag="psJ")

            def emit_pad(cols_total, rhs_tile, j):
                # realize pad-columns of PE work as junk matmuls: full
                # 512-col units plus one fractional column-slice
                while cols_total >= 64:
                    cols = 512 if cols_total >= 512 else cols_total
                    nc.tensor.matmul(
                        psJ[0:2 * E, 0:cols], lhsT=gt[:, 0, :],
                        rhs=rhs_tile[:, j, 0:cols],
                        start=True, stop=True,
                        tile_position=(0, 0),
                        skip_group_check=True)
                    cols_total -= cols

            def seg_body(t0, tn, groups, si):
                nq = tn // 512
                qg0 = t0 // 512
                psA = psl.tile([P, 512], F32, tag="psA")
                c0 = 0
                for gi, g in enumerate(groups):
                    ht = hp.tile([P, 5, 2048], F16, tag="ht")
                    nc.sync.dma_start(out=ht[:, 0:g, 0:tn],
                                      in_=hh[:, c0:c0 + g, t0:t0 + tn])
                    if si == 0 and gi == 1:
                        load_singles()
                    for j in range(g):
                        c = c0 + j
                        for q in range(nq):
                            nc.tensor.matmul(
                                psA[32 * q:32 * q + 2 * E, :],
                                lhsT=gt[:, c, :],
                                rhs=ht[:, j, q * 512:(q + 1) * 512],
                                start=(c == 0), stop=(c == NCH - 1),
                                tile_position=(0, 32 * q),
                                skip_group_check=True)
                        emit_pad(pads["chunk"][si * NCH + c], ht, j)
                    c0 += g
                    last_ht = ht

                # PSUM -> SBUF (PE matmuls read SBUF only); valid rows
                # are the col-group bands 32q..32q+16
                hi = 32 * (nq - 1) + 2 * E
                a16 = big.tile([P, 512], F32, tag="a16")
                nc.vector.tensor_copy(out=a16[0:hi, 0:256],
                                      in_=psA[0:hi, 0:256])
                nc.scalar.copy(out=a16[0:hi, 256:512], in_=psA[0:hi, 256:512])
                # bridge PE over the PSUM-copy latency (combine matmuls
                # wait on a16) so its DVFS run doesn't break here
                emit_pad(pads["boundary"][si], last_ht, 0)

                # combine-transpose: per block b=(q,bj), tokens {4k+bj}:
                #   tp[:, b] = a16[32q:32q+16]^T @ [I; 2^-11 I]
                nb = nq * N_BJ
                tp = pst.tile([P, NG * N_BJ * E], F32, tag="tp")
                for q in range(nq):
                    slA = slice(32 * q, 32 * q + 2 * E)
                    aR = a16[slA, :].rearrange("e (k bj) -> e bj k", bj=N_BJ)
                    for bj in range(N_BJ):
                        b = q * N_BJ + bj
                        nc.tensor.matmul(
                            tp[:, b * E:(b + 1) * E], lhsT=aR[:, bj, :],
                            rhs=mabt[slA, :], start=True, stop=True,
                            tile_position=(32 * q, 0),
                            skip_group_check=True)

                # sc = tp + bias (token-major; bias varies along free dim)
                sc = ep.tile([P, NG * N_BJ, E], F32, tag="sc")
                nc.vector.tensor_tensor(
                    out=sc[:, 0:nb, :], in0=tp[:, 0:nb * E].rearrange(
                        "p (b e) -> p b e", e=E),
                    in1=btile.broadcast_to([P, nb, E]),
                    op=mybir.AluOpType.add)

                mx = ep.tile([P, NG * N_BJ, E], F32, tag="mx")
                mi = ep.tile([P, NG * N_BJ, E], U32, tag="mi")
                for b in range(nb):
                    nc.vector.max(out=mx[:, b, :], in_=sc[:, b, :])

                # top-2 softmax = sigmoid(+-(v0-v1)): one DVE subtract,
                # two ACT sigmoids; the ACT passes overlap the DVE index
                # pass below
                dt_ = ep.tile([P, NG * N_BJ], F32, tag="dt")
                nc.vector.tensor_tensor(
                    out=dt_[:, 0:nb], in0=mx[:, 0:nb, 0], in1=mx[:, 0:nb, 1],
                    op=mybir.AluOpType.subtract)
                ov = ep.tile([P, NG * N_BJ, 4], F32, tag="ov")
                nc.scalar.activation(
                    out=ov[:, 0:nb, 0], in_=dt_[:, 0:nb],
                    func=mybir.ActivationFunctionType.Sigmoid)
                nc.scalar.activation(
                    out=ov[:, 0:nb, 1], in_=dt_[:, 0:nb],
                    func=mybir.ActivationFunctionType.Sigmoid, scale=-1.0)
                for b in range(nb):
                    nc.vector.max_index(out=mi[:, b, :],
                                        in_max=mx[:, b, :],
                                        in_values=sc[:, b, :])
                nc.vector.tensor_copy(out=ov[:, 0:nb, 2:4],
                                      in_=mi[:, 0:nb, 0:2])

                # one merged [w1 w2 e1 e2] store; mid-stream segments go
                # out the ACT ring (their wait would stall the in-order
                # SP h queue), the final one uses the now-idle SP ring
                out_eng = nc.sync if si == len(SEGS) - 1 else nc.scalar
                out_eng.dma_start(
                    out=ob[:, qg0:qg0 + nq], in_=ov[:, 0:nb].rearrange(
                        "k (q bj) u -> k q bj u", q=nq))

            def body():
                for si, (t0, tn, groups) in enumerate(SEGS):
                    seg_body(t0, tn, groups, si)

            if n_rep == 1:
                body()
            else:
                with tc.For_i(0, n_rep, 1):
                    body()

    nc.finalize()
    return nc


def _get_nc():
    if "nc" not in _NC_CACHE:
        _NC_CACHE["nc"] = build_router_nc()
    return _NC_CACHE["nc"]


def make_gate_inputs(pressure_bias, temperature_field, gate_w):
    gw = np.asarray(gate_w, dtype=np.float32)
    pb = np.asarray(pressure_bias, np.float32)
    temp = np.asarray(temperature_field, np.float32)
    it = 1.0 / np.clip(temp, np.float32(0.1), np.float32(10.0))
    gs = gw * it[:, None]
    gT = np.ascontiguousarray(gs.T)                         # [D, E]
    gh = gT.astype(np.float16)
    gl = ((gT - gh.astype(np.float32)) * SC).astype(np.float16)
    gcomb = np.concatenate([gh, gl], axis=1)                # [D, 16]
    g2 = np.ascontiguousarray(
        gcomb.reshape(NCH, P, 2 * E).transpose(1, 0, 2))    # [P, NCH, 16]
    eye = np.eye(E, dtype=np.float32)
    mm = np.zeros((P, E), np.float32)
    for q in range(4):
        mm[32 * q:32 * q + E, :] = eye
        mm[32 * q + E:32 * q + 2 * E, :] = eye * INV_SC
    bias = (pb * it).astype(np.float32)
    bt = np.ascontiguousarray(np.broadcast_to(bias, (P, 1, E)))
    return g2, mm, bt


def make_h_inputs(hs_core):
    hT = np.ascontiguousarray(hs_core.T).astype(np.float16)
    return np.ascontiguousarray(
        hT.reshape(NCH, P, T_CORE).transpose(1, 0, 2))


def unshuffle_out(arr, t_core):
    # arr [P, NG, N_BJ, u]: token = 512*g + 4*k + bj
    return np.ascontiguousarray(
        arr.transpose(1, 0, 2, 3).reshape(t_core, arr.shape[-1]))


def decode_idx(e_f):
    # device writes indices via a u32->f32 tensor_copy; accept either
    # value-convert (0.0..7.0) or raw bitcast (denormal) semantics
    if e_f.size and np.abs(e_f).max() < 1e-6:
        return np.ascontiguousarray(e_f).view(np.uint32).astype(np.int32)
    return np.round(e_f).astype(np.int32)


def kernel(hidden_states, pressure_bias, temperature_field, gate_w):
    hs = np.ascontiguousarray(np.asarray(hidden_states, dtype=np.float32))
    hs = hs.reshape(T_TOTAL, D)
    g2, mm, bt = make_gate_inputs(pressure_bias, temperature_field, gate_w)

    in_maps = []
    for i in range(N_CORES):
        hh_dev = make_h_inputs(hs[i * T_CORE:(i + 1) * T_CORE])
        in_maps.append({"hh": hh_dev, "g2": g2, "mm": mm, "bt": bt})

    nc = _get_nc()
    global LAST_RESULT
    res = run_bass_kernel_spmd(nc, in_maps, core_ids=list(range(N_CORES)),
                               trace=TRACE)
    LAST_RESULT = res

    weights = np.empty((T_TOTAL, 2), np.float32)
    experts = np.empty((T_TOTAL, 2), np.int32)
    for i, r in enumerate(res.results):
        o = unshuffle_out(r["ob"], T_CORE)
        weights[i * T_CORE:(i + 1) * T_CORE] = o[:, 0:2]
        experts[i * T_CORE:(i + 1) * T_CORE] = decode_idx(o[:, 2:4])

    return weights.reshape(B, S, 2), experts.reshape(B, S, 2)
